# revision 17
# baseline (speedup 1.0000x reference)
"""Single-head causal attention (B=256, T=256, E=512, H=64) on 8 trn2 cores.

Strategy (per core, 32 batches, data-parallel over B):
  - x loaded from DRAM with cast-to-bf16 during DMA (SWDGE), then transposed
    e-major via the xbar DMA transpose, split across BOTH HWDGE queues
    (sync + scalar) -> xT [e,t] chunks.
  - Wq (scaled by 1/sqrt(64)) and Wk packed side-by-side into one stationary
    [128e, 128]: a single matmul per e-chunk emits qT (psum partitions 0:64)
    and kT (partitions 64:128) together.
  - v natural [t, h] per slot (xT stationary).
  - weiT[s,t] = kT.T @ qT directly (transposed logits): after exp + causal
    mask this IS the stationary operand the output matmul needs.
    The fully-masked (s-chunk 1, t-chunk 0) block is never computed:
    wei streams 384 cols/batch instead of 512.
  - A ones-column appended to v makes the output matmul produce the masked
    softmax row-sums for free: out_psum[:, 64] = rowsum.
  - No max-subtraction in softmax: logits are ~N(0,1), exp() is safe.
"""

import numpy as np

import concourse.bass as bass
import concourse.mybir as mybir
import concourse.tile as tile
from concourse import bacc
from concourse.bass_utils import run_bass_kernel_spmd

F32 = mybir.dt.float32
BF16 = mybir.dt.bfloat16

B, T, E, H = 256, 256, 512, 64
N_CORES = 8
BPC = B // N_CORES      # 32 batches per core
GRP = 2                 # batches per pipelined group
EC = E // 128           # 4 e-chunks
TT = T // 128           # 2 t-tiles per batch
SLOTS = GRP * TT        # 4 (batch, t-tile) slots per group


def build_kernel(bpc: int = BPC, trace_scopes: bool = False):
    ngrp = bpc // GRP
    nc = bacc.Bacc("TRN2", target_bir_lowering=False, num_devices=N_CORES)

    x = nc.dram_tensor("x", [bpc, T, E], F32, kind="ExternalInput")
    wq = nc.dram_tensor("wq", [H, E], F32, kind="ExternalInput")
    wk = nc.dram_tensor("wk", [H, E], F32, kind="ExternalInput")
    wv = nc.dram_tensor("wv", [H, E], F32, kind="ExternalInput")
    y = nc.dram_tensor("y", [bpc, T, H], F32, kind="ExternalOutput")

    with tile.TileContext(nc) as tc:
        with (
            tc.tile_pool(name="const", bufs=1) as constp,
            tc.tile_pool(name="wprep", bufs=1) as wprep,
            tc.tile_pool(name="xload", bufs=4) as xloadp,
            tc.tile_pool(name="xtp", bufs=4) as xtp,
            tc.tile_pool(name="qkv", bufs=2) as qkvp,
            tc.tile_pool(name="ptile", bufs=3) as ptp,
            tc.tile_pool(name="outs", bufs=3) as outp,
            tc.tile_pool(name="psqk", bufs=1, space="PSUM") as psqkp,
            tc.tile_pool(name="psv", bufs=1, space="PSUM") as psvp,
            tc.tile_pool(name="ptr", bufs=2, space="PSUM") as ptrp,
            tc.tile_pool(name="psw", bufs=2, space="PSUM") as pswp,
            tc.tile_pool(name="pso", bufs=2, space="PSUM") as psop,
        ):
            # ---- weight DMAs first (tiny; they'd starve behind the 1MB
            # x loads otherwise), then prologue x loads, then weight compute.
            wspec = (
                ("q", wq, H ** -0.5),
                ("k", wk, 1.0),
                ("v", wv, 1.0),
            )
            wfs = {}
            for name, wdram, _ in wspec:
                wf = wprep.tile([H, E], F32, tag=f"wf{name}")
                nc.sync.dma_start(wf[:], wdram[:])
                wfs[name] = wf

            prologue_loads = []
            for g0_ in range(min(3, ngrp)):
                xb_ = xloadp.tile([128, SLOTS, E], BF16, tag="xb")
                nc.gpsimd.dma_start(
                    xb_[:],
                    x[g0_ * GRP : (g0_ + 1) * GRP].rearrange(
                        "b (j p) e -> p (b j) e", p=128
                    ),
                )
                prologue_loads.append((g0_, xb_))

            # identity (bf16) for PE is_transpose matmuls
            ident = constp.tile([128, 128], BF16, tag="ident")
            ones = wprep.tile([128, 128], BF16, tag="ones")
            nc.vector.memset(ones[:], 1.0)
            nc.gpsimd.affine_select(
                out=ident[:], in_=ones[:],
                compare_op=mybir.AluOpType.is_equal, fill=0.0,
                base=0, channel_multiplier=-1, pattern=[[1, 128]],
            )

            # ---- weight prep compute (one-time), transposes on the PE so
            # the prologue x loads never contend with xbar traffic ----
            # wqkT: [128 (e within chunk), EC, 128] bf16; cols 0:64 = WqT
            # (softmax scale folded in), cols 64:128 = WkT.
            wqkT = constp.tile([128, EC, 2 * H], BF16, tag="wqkT")
            wvT = constp.tile([128, EC, H], BF16, tag="wvT")
            wdst = {"q": wqkT[:, :, 0:H], "k": wqkT[:, :, H : 2 * H], "v": wvT[:]}
            for name, _, scale in wspec:
                wf, dst = wfs[name], wdst[name]
                wb = wprep.tile([H, E], BF16, tag=f"wb{name}")
                nc.scalar.activation(wb[:], wf[:], mybir.ActivationFunctionType.Copy, scale=float(scale))
                pw = ptrp.tile([128, EC, 128], BF16, tag="ptr")
                for c in range(EC):
                    nc.tensor.transpose(
                        pw[:, c, 0:H], wb[:, c * 128 : (c + 1) * 128], ident[0:H, 0:H]
                    )
                nc.scalar.activation(dst, pw[:, :, 0:H], mybir.ActivationFunctionType.Copy)

            # ---- software-pipelined main loop (1-group skew) ----
            # sync queue carries ONLY the transposes so transpose(g+1) is
            # never trapped behind group-g compute; y-writes go via scalar.
            # Tensor order per iteration: psw(g-1), psqk(g)+psv(g), pso(g-1)
            # so the exp/mask latency of g-1 hides under g's projections.
            NQK = 128 * SLOTS
            state = {}   # g -> (qkT, v1)
            pts = {}     # g -> [PT, PT]

            def load_x(g):
                b0 = g * GRP
                # load GRP batches of x, casting f32->bf16 during DMA (SWDGE)
                xb = xloadp.tile([128, SLOTS, E], BF16, tag="xb")
                nc.gpsimd.dma_start(
                    xb[:],
                    x[b0 : b0 + GRP].rearrange("b (j p) e -> p (b j) e", p=128),
                )
                return xb

            def pe_transpose(g, xb):
                # transpose on the PE (xbar DMA transposes contend with the
                # x loads for DMA-engine bandwidth - measured 2x slowdown);
                # is_transpose matmuls + scalar PSUM->SBUF copies instead.
                xT = xtp.tile([128, SLOTS, EC, 128], BF16, tag="xT")
                for s in range(SLOTS):
                    ptr = ptrp.tile([128, EC, 128], BF16, tag="ptr")
                    for c in range(EC):
                        nc.tensor.transpose(
                            ptr[:, c, :], xb[:, s, c * 128 : (c + 1) * 128], ident[:]
                        )
                    # balance PSUM->SBUF copies across scalar and vector
                    if s % 2 == 0:
                        nc.scalar.activation(
                            xT[:, s, :, :], ptr[:], mybir.ActivationFunctionType.Copy
                        )
                    else:
                        nc.vector.tensor_copy(xT[:, s, :, :], ptr[:])
                return xT

            def project(g, xT):
                # packed q+k projection: one matmul per e-chunk,
                # psqk partitions 0:64 = qT, 64:128 = kT; N = 128*SLOTS
                psqk = psqkp.tile([2 * H, NQK], F32, tag="psqk")
                for c in range(EC):
                    nc.tensor.matmul(
                        psqk[:], wqkT[:, c, :], xT[:, :, c, :],
                        start=(c == 0), stop=(c == EC - 1),
                    )
                # v natural [t, h] per slot (xT stationary)
                psv = psvp.tile([128, SLOTS, H], F32, tag="psv")
                for s in range(SLOTS):
                    for c in range(EC):
                        nc.tensor.matmul(
                            psv[:, s, :], xT[:, s, c, :], wvT[:, c, :],
                            start=(c == 0), stop=(c == EC - 1),
                        )
                # cast to bf16; kT's copy shifts partitions 64:128 -> 0:64 so
                # both wei-matmul operands share base partition 0.
                # qkT cols 0:512 = qT, cols 512:1024 = kT.
                qkT = qkvp.tile([H, 2 * NQK], BF16, tag="qkT")
                nc.vector.tensor_copy(qkT[:, 0:NQK], psqk[0:H, :])
                nc.vector.tensor_copy(qkT[:, NQK : 2 * NQK], psqk[H : 2 * H, :])
                v1 = qkvp.tile([128, SLOTS, H + 1], BF16, tag="v1")
                nc.vector.tensor_copy(v1[:, :, 0:H], psv[:])
                nc.vector.memset(v1[:, :, H : H + 1], 1.0)
                state[g] = (qkT, v1)

            def attn_logits(g):
                # wei + exp + mask, skipping the fully-masked (sc=1, tt=0)
                # block. psw layout [128, T+128]: cols 0:256 = sc0 (all t),
                # cols 256:384 = sc1 (t in 128:256).
                qkT, _ = state[g]
                pts[g] = []
                for b2 in range(GRP):
                    tbase = b2 * T
                    psw = pswp.tile([128, T + 128], F32, tag="psw")
                    nc.tensor.matmul(
                        psw[:, 0:T],
                        qkT[:, NQK + tbase : NQK + tbase + 128],
                        qkT[:, tbase : tbase + T],
                        start=True, stop=True,
                    )
                    nc.tensor.matmul(
                        psw[:, T : T + 128],
                        qkT[:, NQK + tbase + 128 : NQK + tbase + T],
                        qkT[:, tbase + 128 : tbase + T],
                        start=True, stop=True,
                    )
                    PT = ptp.tile([128, T + 128], BF16, tag="PT")
                    nc.scalar.activation(PT[:], psw[:], mybir.ActivationFunctionType.Exp)
                    # causal mask only on the two diagonal blocks:
                    # keep col j >= partition p (s_local <= t_local)
                    for cb in (0, T):
                        nc.gpsimd.affine_select(
                            out=PT[:, cb : cb + 128],
                            in_=PT[:, cb : cb + 128],
                            compare_op=mybir.AluOpType.is_ge,
                            fill=0.0,
                            base=0,
                            channel_multiplier=-1,
                            pattern=[[1, 128]],
                        )
                    pts[g].append(PT)

            def attn_out(g):
                b0 = g * GRP
                _, v1 = state[g]
                for b2 in range(GRP):
                    PT = pts[g][b2]
                    # out[t, 0:H] = P @ v ; out[t, H] = rowsum (ones column)
                    pso = psop.tile([128, TT, H + 1], F32, tag="pso")
                    nc.tensor.matmul(
                        pso[:, 0, :], PT[:, 0:128], v1[:, b2 * TT + 0, :],
                        start=True, stop=True,
                    )
                    nc.tensor.matmul(
                        pso[:, 1, :], PT[:, 128:256], v1[:, b2 * TT + 0, :],
                        start=True, stop=False,
                    )
                    nc.tensor.matmul(
                        pso[:, 1, :], PT[:, T : T + 128], v1[:, b2 * TT + 1, :],
                        start=False, stop=True,
                    )
                    rec = outp.tile([128, TT, 1], F32, tag="rec")
                    nc.vector.reciprocal(rec[:], pso[:, :, H : H + 1])
                    ob = outp.tile([128, TT, H], F32, tag="ob")
                    for tt in range(TT):
                        nc.vector.tensor_scalar_mul(
                            ob[:, tt, :], pso[:, tt, 0:H], rec[:, tt, :]
                        )
                    nc.sync.dma_start(
                        y[b0 + b2].rearrange("(tt p) h -> p tt h", p=128),
                        ob[:],
                    )
                del state[g], pts[g]

            # transposes run one group ahead of projections so the scalar
            # PSUM->SBUF copies have a full iteration of slack before
            # psqk(g) consumes xT(g).
            LOOKAHEAD = 3
            xbs = {g: xb for g, xb in prologue_loads}
            xTs = {0: pe_transpose(0, xbs.pop(0))}
            for g in range(ngrp):
                if g + LOOKAHEAD < ngrp:
                    xbs[g + LOOKAHEAD] = load_x(g + LOOKAHEAD)
                if g >= 1:
                    attn_logits(g - 1)
                if g + 1 < ngrp:
                    xTs[g + 1] = pe_transpose(g + 1, xbs.pop(g + 1))
                project(g, xTs.pop(g))
                if g >= 1:
                    attn_out(g - 1)
            attn_logits(ngrp - 1)
            attn_out(ngrp - 1)

    nc.finalize()
    return nc


_NC_CACHE = {}


def _get_nc(bpc: int = BPC):
    if bpc not in _NC_CACHE:
        _NC_CACHE[bpc] = build_kernel(bpc)
    return _NC_CACHE[bpc]


def kernel(x, Wk, Wq, Wv, _trace: bool = False, _bpc: int = BPC):
    """Full inputs in, full output out. Shards batch dim over 8 cores."""
    x = np.ascontiguousarray(x, dtype=np.float32)
    Wk = np.ascontiguousarray(Wk, dtype=np.float32)
    Wq = np.ascontiguousarray(Wq, dtype=np.float32)
    Wv = np.ascontiguousarray(Wv, dtype=np.float32)
    nb = x.shape[0]
    bpc = nb // N_CORES
    nc = _get_nc(bpc)
    in_maps = [
        {"x": x[i * bpc : (i + 1) * bpc], "wq": Wq, "wk": Wk, "wv": Wv}
        for i in range(N_CORES)
    ]
    res = run_bass_kernel_spmd(
        nc, in_maps, core_ids=list(range(N_CORES)), trace=_trace
    )
    out = np.concatenate([res.results[i]["y"] for i in range(N_CORES)], axis=0)
    if _trace:
        kernel.last_results = res
    return out


# revision 18
# speedup vs baseline: 1.0565x; 1.0565x over previous
"""Single-head causal attention (B=256, T=256, E=512, H=64) on 8 trn2 cores.

Strategy (per core, 32 batches, data-parallel over B):
  - x loaded from DRAM with cast-to-bf16 during DMA (SWDGE), then transposed
    e-major via the xbar DMA transpose, split across BOTH HWDGE queues
    (sync + scalar) -> xT [e,t] chunks.
  - Wq (scaled by 1/sqrt(64)) and Wk packed side-by-side into one stationary
    [128e, 128]: a single matmul per e-chunk emits qT (psum partitions 0:64)
    and kT (partitions 64:128) together.
  - v natural [t, h] per slot (xT stationary).
  - weiT[s,t] = kT.T @ qT directly (transposed logits): after exp + causal
    mask this IS the stationary operand the output matmul needs.
    The fully-masked (s-chunk 1, t-chunk 0) block is never computed:
    wei streams 384 cols/batch instead of 512.
  - A ones-column appended to v makes the output matmul produce the masked
    softmax row-sums for free: out_psum[:, 64] = rowsum.
  - No max-subtraction in softmax: logits are ~N(0,1), exp() is safe.
"""

import numpy as np

import concourse.bass as bass
import concourse.mybir as mybir
import concourse.tile as tile
from concourse import bacc
from concourse.bass_utils import run_bass_kernel_spmd

F32 = mybir.dt.float32
BF16 = mybir.dt.bfloat16

B, T, E, H = 256, 256, 512, 64
N_CORES = 8
BPC = B // N_CORES      # 32 batches per core
GRP = 2                 # batches per pipelined group
EC = E // 128           # 4 e-chunks
TT = T // 128           # 2 t-tiles per batch
SLOTS = GRP * TT        # 4 (batch, t-tile) slots per group


def build_kernel(bpc: int = BPC, trace_scopes: bool = False):
    ngrp = bpc // GRP
    nc = bacc.Bacc("TRN2", target_bir_lowering=False, num_devices=N_CORES)

    x = nc.dram_tensor("x", [bpc, T, E], F32, kind="ExternalInput")
    wq = nc.dram_tensor("wq", [H, E], F32, kind="ExternalInput")
    wk = nc.dram_tensor("wk", [H, E], F32, kind="ExternalInput")
    wv = nc.dram_tensor("wv", [H, E], F32, kind="ExternalInput")
    y = nc.dram_tensor("y", [bpc, T, H], F32, kind="ExternalOutput")

    with tile.TileContext(nc) as tc:
        with (
            tc.tile_pool(name="const", bufs=1) as constp,
            tc.tile_pool(name="wprep", bufs=1) as wprep,
            tc.tile_pool(name="xload", bufs=4) as xloadp,
            tc.tile_pool(name="xtp", bufs=4) as xtp,
            tc.tile_pool(name="qkv", bufs=2) as qkvp,
            tc.tile_pool(name="ptile", bufs=3) as ptp,
            tc.tile_pool(name="outs", bufs=3) as outp,
            tc.tile_pool(name="psqk", bufs=1, space="PSUM") as psqkp,
            tc.tile_pool(name="psv", bufs=1, space="PSUM") as psvp,
            tc.tile_pool(name="ptr", bufs=2, space="PSUM") as ptrp,
            tc.tile_pool(name="psw", bufs=2, space="PSUM") as pswp,
            tc.tile_pool(name="pso", bufs=2, space="PSUM") as psop,
        ):
            # ---- weight DMAs first (tiny; they'd starve behind the 1MB
            # x loads otherwise), then prologue x loads, then weight compute.
            wspec = (
                ("q", wq, H ** -0.5),
                ("k", wk, 1.0),
                ("v", wv, 1.0),
            )
            wfs = {}
            for name, wdram, _ in wspec:
                wf = wprep.tile([H, E], F32, tag=f"wf{name}")
                nc.sync.dma_start(wf[:], wdram[:])
                wfs[name] = wf

            prologue_loads = []
            for g0_ in range(min(3, ngrp)):
                xb_ = xloadp.tile([128, GRP, 2, E], BF16, tag="xb")
                nc.gpsimd.dma_start(
                    xb_[:],
                    x[g0_ * GRP : (g0_ + 1) * GRP].rearrange(
                        "b (p j) e -> p b j e", j=2
                    ),
                )
                prologue_loads.append((g0_, xb_))

            # identity (bf16) for PE is_transpose matmuls
            ident = constp.tile([128, 128], BF16, tag="ident")
            ones = wprep.tile([128, 128], BF16, tag="ones")
            nc.vector.memset(ones[:], 1.0)
            nc.gpsimd.affine_select(
                out=ident[:], in_=ones[:],
                compare_op=mybir.AluOpType.is_equal, fill=0.0,
                base=0, channel_multiplier=-1, pattern=[[1, 128]],
            )

            # ---- weight prep compute (one-time), transposes on the PE so
            # the prologue x loads never contend with xbar traffic ----
            # wqkT: [128 (e within chunk), EC, 128] bf16; cols 0:64 = WqT
            # (softmax scale folded in), cols 64:128 = WkT.
            wqkT = constp.tile([128, EC, 2 * H], BF16, tag="wqkT")
            wvT = constp.tile([128, EC, H], BF16, tag="wvT")
            wdst = {"q": wqkT[:, :, 0:H], "k": wqkT[:, :, H : 2 * H], "v": wvT[:]}
            for name, _, scale in wspec:
                wf, dst = wfs[name], wdst[name]
                wb = wprep.tile([H, E], BF16, tag=f"wb{name}")
                nc.scalar.activation(wb[:], wf[:], mybir.ActivationFunctionType.Copy, scale=float(scale))
                pw = ptrp.tile([128, EC, 128], BF16, tag="ptr")
                for c in range(EC):
                    nc.tensor.transpose(
                        pw[:, c, 0:H], wb[:, c * 128 : (c + 1) * 128], ident[0:H, 0:H]
                    )
                nc.scalar.activation(dst, pw[:, :, 0:H], mybir.ActivationFunctionType.Copy)

            # ---- software-pipelined main loop (1-group skew) ----
            # sync queue carries ONLY the transposes so transpose(g+1) is
            # never trapped behind group-g compute; y-writes go via scalar.
            # Tensor order per iteration: psw(g-1), psqk(g)+psv(g), pso(g-1)
            # so the exp/mask latency of g-1 hides under g's projections.
            NQK = 128 * SLOTS
            state = {}   # g -> (qkT, v1)
            pts = {}     # g -> [PT, PT]

            def load_x(g):
                b0 = g * GRP
                # load GRP batches of x, casting f32->bf16 during DMA (SWDGE).
                # interleave-2 layout: slot (b, j) holds tokens t = 2p + j,
                # giving 4KB-contiguous load descriptors (2 consecutive t rows)
                # and 512B y-write descriptors instead of 256B.
                xb = xloadp.tile([128, GRP, 2, E], BF16, tag="xb")
                nc.gpsimd.dma_start(
                    xb[:],
                    x[b0 : b0 + GRP].rearrange("b (p j) e -> p b j e", j=2),
                )
                return xb

            def pe_transpose(g, xb):
                # transpose on the PE (xbar DMA transposes contend with the
                # x loads for DMA-engine bandwidth - measured 2x slowdown);
                # is_transpose matmuls + scalar PSUM->SBUF copies instead.
                xT = xtp.tile([128, SLOTS, EC, 128], BF16, tag="xT")
                for s in range(SLOTS):
                    ptr = ptrp.tile([128, EC, 128], BF16, tag="ptr")
                    for c in range(EC):
                        nc.tensor.transpose(
                            ptr[:, c, :],
                            xb[:, s // 2, s % 2, c * 128 : (c + 1) * 128],
                            ident[:],
                        )
                    # balance PSUM->SBUF copies across scalar and vector
                    if s % 2 == 0:
                        nc.scalar.activation(
                            xT[:, s, :, :], ptr[:], mybir.ActivationFunctionType.Copy
                        )
                    else:
                        nc.vector.tensor_copy(xT[:, s, :, :], ptr[:])
                return xT

            def project(g, xT):
                # packed q+k projection: one matmul per e-chunk,
                # psqk partitions 0:64 = qT, 64:128 = kT; N = 128*SLOTS
                psqk = psqkp.tile([2 * H, NQK], F32, tag="psqk")
                for c in range(EC):
                    nc.tensor.matmul(
                        psqk[:], wqkT[:, c, :], xT[:, :, c, :],
                        start=(c == 0), stop=(c == EC - 1),
                    )
                # v natural [t, h] per slot (xT stationary)
                psv = psvp.tile([128, SLOTS, H], F32, tag="psv")
                for s in range(SLOTS):
                    for c in range(EC):
                        nc.tensor.matmul(
                            psv[:, s, :], xT[:, s, c, :], wvT[:, c, :],
                            start=(c == 0), stop=(c == EC - 1),
                        )
                # cast to bf16; kT's copy shifts partitions 64:128 -> 0:64 so
                # both wei-matmul operands share base partition 0.
                # qkT cols 0:512 = qT, cols 512:1024 = kT.
                qkT = qkvp.tile([H, 2 * NQK], BF16, tag="qkT")
                nc.vector.tensor_copy(qkT[:, 0:NQK], psqk[0:H, :])
                nc.vector.tensor_copy(qkT[:, NQK : 2 * NQK], psqk[H : 2 * H, :])
                v1 = qkvp.tile([128, SLOTS, H + 1], BF16, tag="v1")
                nc.vector.tensor_copy(v1[:, :, 0:H], psv[:])
                nc.vector.memset(v1[:, :, H : H + 1], 1.0)
                state[g] = (qkT, v1)

            def attn_logits(g):
                # wei + exp + mask in interleave-2 space: s = 2sp+js,
                # t = 2tc+jt. psw block (js, jt) at plane 2*js+jt.
                qkT, _ = state[g]
                pts[g] = []
                for b2 in range(GRP):
                    sb = b2 * 2
                    psw = pswp.tile([128, 4, 128], F32, tag="psw")
                    for js in range(2):
                        for jt in range(2):
                            nc.tensor.matmul(
                                psw[:, 2 * js + jt, :],
                                qkT[:, NQK + (sb + js) * 128 : NQK + (sb + js + 1) * 128],
                                qkT[:, (sb + jt) * 128 : (sb + jt + 1) * 128],
                                start=True, stop=True,
                            )
                    PT = ptp.tile([128, 4, 128], BF16, tag="PT")
                    nc.scalar.activation(PT[:], psw[:], mybir.ActivationFunctionType.Exp)
                    # keep 2sp+js <= 2tc+jt: blocks (0,0),(0,1),(1,1) keep
                    # tc >= sp; block (1,0) keeps tc >= sp+1
                    for js in range(2):
                        for jt in range(2):
                            nc.gpsimd.affine_select(
                                out=PT[:, 2 * js + jt, :],
                                in_=PT[:, 2 * js + jt, :],
                                compare_op=mybir.AluOpType.is_ge,
                                fill=0.0,
                                base=(-1 if (js == 1 and jt == 0) else 0),
                                channel_multiplier=-1,
                                pattern=[[1, 128]],
                            )
                    pts[g].append(PT)

            def attn_out(g):
                b0 = g * GRP
                _, v1 = state[g]
                for b2 in range(GRP):
                    PT = pts[g][b2]
                    sb = b2 * 2
                    # out[t, 0:H] = P @ v ; out[t, H] = rowsum (ones column)
                    pso = psop.tile([128, 2, H + 1], F32, tag="pso")
                    for jt in range(2):
                        for js in range(2):
                            nc.tensor.matmul(
                                pso[:, jt, :],
                                PT[:, 2 * js + jt, :],
                                v1[:, sb + js, :],
                                start=(js == 0), stop=(js == 1),
                            )
                    rec = outp.tile([128, 2, 1], F32, tag="rec")
                    nc.vector.reciprocal(rec[:], pso[:, :, H : H + 1])
                    ob = outp.tile([128, 2, H], F32, tag="ob")
                    for jt in range(2):
                        nc.vector.tensor_scalar_mul(
                            ob[:, jt, :], pso[:, jt, 0:H], rec[:, jt, :]
                        )
                    nc.sync.dma_start(
                        y[b0 + b2].rearrange("(p j) h -> p j h", j=2),
                        ob[:],
                    )
                del state[g], pts[g]

            # transposes run one group ahead of projections so the scalar
            # PSUM->SBUF copies have a full iteration of slack before
            # psqk(g) consumes xT(g).
            LOOKAHEAD = 3
            xbs = {g: xb for g, xb in prologue_loads}
            xTs = {0: pe_transpose(0, xbs.pop(0))}
            for g in range(ngrp):
                if g + LOOKAHEAD < ngrp:
                    xbs[g + LOOKAHEAD] = load_x(g + LOOKAHEAD)
                if g >= 1:
                    attn_logits(g - 1)
                if g + 1 < ngrp:
                    xTs[g + 1] = pe_transpose(g + 1, xbs.pop(g + 1))
                project(g, xTs.pop(g))
                if g >= 1:
                    attn_out(g - 1)
            attn_logits(ngrp - 1)
            attn_out(ngrp - 1)

    nc.finalize()
    return nc


_NC_CACHE = {}


def _get_nc(bpc: int = BPC):
    if bpc not in _NC_CACHE:
        _NC_CACHE[bpc] = build_kernel(bpc)
    return _NC_CACHE[bpc]


def kernel(x, Wk, Wq, Wv, _trace: bool = False, _bpc: int = BPC):
    """Full inputs in, full output out. Shards batch dim over 8 cores."""
    x = np.ascontiguousarray(x, dtype=np.float32)
    Wk = np.ascontiguousarray(Wk, dtype=np.float32)
    Wq = np.ascontiguousarray(Wq, dtype=np.float32)
    Wv = np.ascontiguousarray(Wv, dtype=np.float32)
    nb = x.shape[0]
    bpc = nb // N_CORES
    nc = _get_nc(bpc)
    in_maps = [
        {"x": x[i * bpc : (i + 1) * bpc], "wq": Wq, "wk": Wk, "wv": Wv}
        for i in range(N_CORES)
    ]
    res = run_bass_kernel_spmd(
        nc, in_maps, core_ids=list(range(N_CORES)), trace=_trace
    )
    out = np.concatenate([res.results[i]["y"] for i in range(N_CORES)], axis=0)
    if _trace:
        kernel.last_results = res
    return out


# revision 26
# speedup vs baseline: 1.1475x; 1.0862x over previous
"""Single-head causal attention (B=256, T=256, E=512, H=64) on 8 trn2 cores.

Strategy (per core, 32 batches, data-parallel over B):
  - Interleave-2 token layout: partition p holds tokens t = 2p+j (j in 0,1),
    so x loads use 4KB-contiguous descriptors and y writes 512B ones
    (256B descriptors made the y path 60GB/s and DMA the wall).
  - x cast f32->bf16 during the SWDGE load; transposed e-major ON THE PE
    (is_transpose matmuls + scalar/vector PSUM->SBUF copies). The xbar DMA
    transpose contends with the x loads for DMA bandwidth (measured 2x
    degradation, additive) and must never run from both HWDGE queues
    concurrently (corrupts data) - the PE path avoids all of that.
  - Wq (scaled by 1/sqrt(64)) and Wk packed side-by-side into one stationary
    [128e, 128]: a single matmul per e-chunk emits qT (psum partitions 0:64)
    and kT (partitions 64:128) together; the bf16 cast shifts kT down to
    base partition 0 (matmul operands must share a base partition).
  - weiT[s,t] = kT.T @ qT directly (transposed logits): after exp + causal
    mask this IS the stationary operand the output matmul needs. Masks are
    affine per (js, jt) interleave block: keep tc >= sp (+1 for js>jt).
  - A ones-column appended to v makes the output matmul produce the masked
    softmax row-sums for free: out_psum[:, 64] = rowsum.
  - Software-pipelined with a 1-group skew (transposes a further group
    ahead, loads 3 ahead); per-iteration tensor order
    psw(g-1), T(g+1), psqk/psv(g), pso(g-1) hides the exp/mask latency.
  - No max-subtraction in softmax: logits are ~N(0,1), exp() is safe.
"""

import numpy as np

import concourse.bass as bass
import concourse.mybir as mybir
import concourse.tile as tile
from concourse import bacc
from concourse.bass_utils import run_bass_kernel_spmd

F32 = mybir.dt.float32
BF16 = mybir.dt.bfloat16

B, T, E, H = 256, 256, 512, 64
N_CORES = 8
BPC = B // N_CORES      # 32 batches per core
GRP = 2                 # batches per pipelined group
EC = E // 128           # 4 e-chunks
TT = T // 128           # 2 t-tiles per batch
SLOTS = GRP * TT        # 4 (batch, t-tile) slots per group


def build_kernel(bpc: int = BPC, trace_scopes: bool = False):
    ngrp = bpc // GRP
    nc = bacc.Bacc("TRN2", target_bir_lowering=False, num_devices=N_CORES)

    x = nc.dram_tensor("x", [bpc, T, E], F32, kind="ExternalInput")
    wq = nc.dram_tensor("wq", [H, E], F32, kind="ExternalInput")
    wk = nc.dram_tensor("wk", [H, E], F32, kind="ExternalInput")
    wv = nc.dram_tensor("wv", [H, E], F32, kind="ExternalInput")
    y = nc.dram_tensor("y", [bpc, T, H], F32, kind="ExternalOutput")

    with tile.TileContext(nc) as tc:
        with (
            tc.tile_pool(name="const", bufs=1) as constp,
            tc.tile_pool(name="wprep", bufs=1) as wprep,
            tc.tile_pool(name="xload", bufs=5) as xloadp,
            tc.tile_pool(name="xtp", bufs=5) as xtp,
            tc.tile_pool(name="qkv", bufs=3) as qkvp,
            tc.tile_pool(name="ptile", bufs=4) as ptp,
            tc.tile_pool(name="outs", bufs=4) as outp,
            tc.tile_pool(name="psqk", bufs=1, space="PSUM") as psqkp,
            tc.tile_pool(name="psv", bufs=1, space="PSUM") as psvp,
            tc.tile_pool(name="ptr", bufs=2, space="PSUM") as ptrp,
            tc.tile_pool(name="psw", bufs=2, space="PSUM") as pswp,
            tc.tile_pool(name="pso", bufs=2, space="PSUM") as psop,
        ):
            # ---- weight DMAs first (tiny; they'd starve behind the 1MB
            # x loads otherwise), then prologue x loads, then weight compute.
            wspec = (
                ("q", wq, H ** -0.5),
                ("k", wk, 1.0),
                ("v", wv, 1.0),
            )
            wfs = {}
            for name, wdram, _ in wspec:
                wf = wprep.tile([H, E], F32, tag=f"wf{name}")
                nc.sync.dma_start(wf[:], wdram[:])
                wfs[name] = wf

            prologue_loads = []
            for g0_ in range(min(4, ngrp)):
                xb_ = xloadp.tile([128, GRP, 2, E], BF16, tag="xb")
                if g0_ == 0:
                    # group 0 split per slot: the first PE transposes start
                    # after 1/4 of the data instead of the whole group
                    for b_ in range(GRP):
                        for j_ in range(2):
                            nc.gpsimd.dma_start(
                                xb_[:, b_, j_, :],
                                x[b_].rearrange("(p j) e -> p j e", j=2)[:, j_, :],
                            )
                else:
                    nc.gpsimd.dma_start(
                        xb_[:],
                        x[g0_ * GRP : (g0_ + 1) * GRP].rearrange(
                            "b (p j) e -> p b j e", j=2
                        ),
                    )
                prologue_loads.append((g0_, xb_))

            # identity (bf16) for PE is_transpose matmuls
            ident = constp.tile([128, 128], BF16, tag="ident")
            ones = wprep.tile([128, 128], BF16, tag="ones")
            nc.vector.memset(ones[:], 1.0)
            nc.gpsimd.affine_select(
                out=ident[:], in_=ones[:],
                compare_op=mybir.AluOpType.is_equal, fill=0.0,
                base=0, channel_multiplier=-1, pattern=[[1, 128]],
            )

            # ---- weight prep compute (one-time), transposes on the PE so
            # the prologue x loads never contend with xbar traffic ----
            # wqkT: [128 (e within chunk), EC, 128] bf16; cols 0:64 = WqT
            # (softmax scale folded in), cols 64:128 = WkT.
            wqkT = constp.tile([128, EC, 2 * H], BF16, tag="wqkT")
            wvT = constp.tile([128, EC, H], BF16, tag="wvT")
            wdst = {"q": wqkT[:, :, 0:H], "k": wqkT[:, :, H : 2 * H], "v": wvT[:]}
            for name, _, scale in wspec:
                wf, dst = wfs[name], wdst[name]
                wb = wprep.tile([H, E], BF16, tag=f"wb{name}")
                nc.scalar.activation(wb[:], wf[:], mybir.ActivationFunctionType.Copy, scale=float(scale))
                pw = ptrp.tile([128, EC, 128], BF16, tag="ptr")
                for c in range(EC):
                    nc.tensor.transpose(
                        pw[:, c, 0:H], wb[:, c * 128 : (c + 1) * 128], ident[0:H, 0:H]
                    )
                nc.scalar.activation(dst, pw[:, :, 0:H], mybir.ActivationFunctionType.Copy)

            # ---- software-pipelined main loop (1-group skew) ----
            # sync queue carries ONLY the transposes so transpose(g+1) is
            # never trapped behind group-g compute; y-writes go via scalar.
            # Tensor order per iteration: psw(g-1), psqk(g)+psv(g), pso(g-1)
            # so the exp/mask latency of g-1 hides under g's projections.
            NQK = 128 * SLOTS
            state = {}   # g -> (qkT, v1)
            pts = {}     # g -> [PT, PT]

            def load_x(g):
                b0 = g * GRP
                # load GRP batches of x, casting f32->bf16 during DMA (SWDGE).
                # interleave-2 layout: slot (b, j) holds tokens t = 2p + j,
                # giving 4KB-contiguous load descriptors (2 consecutive t rows)
                # and 512B y-write descriptors instead of 256B.
                xb = xloadp.tile([128, GRP, 2, E], BF16, tag="xb")
                nc.gpsimd.dma_start(
                    xb[:],
                    x[b0 : b0 + GRP].rearrange("b (p j) e -> p b j e", j=2),
                )
                return xb

            def pe_transpose(g, xb):
                # transpose on the PE (xbar DMA transposes contend with the
                # x loads for DMA-engine bandwidth - measured 2x slowdown);
                # is_transpose matmuls + scalar PSUM->SBUF copies instead.
                xT = xtp.tile([128, SLOTS, EC, 128], BF16, tag="xT")
                for s in range(SLOTS):
                    ptr = ptrp.tile([128, EC, 128], BF16, tag="ptr")
                    for c in range(EC):
                        nc.tensor.transpose(
                            ptr[:, c, :],
                            xb[:, s // 2, s % 2, c * 128 : (c + 1) * 128],
                            ident[:],
                        )
                    # balance PSUM->SBUF copies across scalar and vector
                    # (slot 3 split halfway to even out engine busy time)
                    if s % 2 == 0:
                        nc.scalar.activation(
                            xT[:, s, :, :], ptr[:], mybir.ActivationFunctionType.Copy
                        )
                    elif s == 1:
                        nc.vector.tensor_copy(xT[:, s, :, :], ptr[:])
                    else:
                        nc.vector.tensor_copy(xT[:, s, 0:2, :], ptr[:, 0:2, :])
                        nc.scalar.activation(
                            xT[:, s, 2:4, :], ptr[:, 2:4, :],
                            mybir.ActivationFunctionType.Copy,
                        )
                return xT

            def project(g, xT):
                # packed q+k projection: one matmul per e-chunk,
                # psqk partitions 0:64 = qT, 64:128 = kT; N = 128*SLOTS
                psqk = psqkp.tile([2 * H, NQK], F32, tag="psqk")
                for c in range(EC):
                    nc.tensor.matmul(
                        psqk[:], wqkT[:, c, :], xT[:, :, c, :],
                        start=(c == 0), stop=(c == EC - 1),
                    )
                # v natural [t, h] per slot (xT stationary)
                psv = psvp.tile([128, SLOTS, H], F32, tag="psv")
                for s in range(SLOTS):
                    for c in range(EC):
                        nc.tensor.matmul(
                            psv[:, s, :], xT[:, s, c, :], wvT[:, c, :],
                            start=(c == 0), stop=(c == EC - 1),
                        )
                # cast to bf16; kT's copy shifts partitions 64:128 -> 0:64 so
                # both wei-matmul operands share base partition 0.
                # qkT cols 0:512 = qT, cols 512:1024 = kT.
                qkT = qkvp.tile([H, 2 * NQK], BF16, tag="qkT")
                nc.vector.tensor_copy(qkT[:, 0:NQK], psqk[0:H, :])
                nc.vector.tensor_copy(qkT[:, NQK : 2 * NQK], psqk[H : 2 * H, :])
                v1 = qkvp.tile([128, SLOTS, H + 1], BF16, tag="v1")
                nc.scalar.activation(v1[:, :, 0:H], psv[:], mybir.ActivationFunctionType.Copy)
                nc.vector.memset(v1[:, :, H : H + 1], 1.0)
                state[g] = (qkT, v1)

            def attn_logits(g):
                # wei + exp + mask in interleave-2 space: s = 2sp+js,
                # t = 2tc+jt. psw block (js, jt) at plane 2*js+jt.
                qkT, _ = state[g]
                pts[g] = []
                for b2 in range(GRP):
                    sb = b2 * 2
                    psw = pswp.tile([128, 4, 128], F32, tag="psw")
                    # blocks (js, jt=0..1) are adjacent planes and share the
                    # kT stationary: one N=256 matmul per js
                    for js in range(2):
                        nc.tensor.matmul(
                            psw[:, 2 * js : 2 * js + 2, :],
                            qkT[:, NQK + (sb + js) * 128 : NQK + (sb + js + 1) * 128],
                            qkT[:, sb * 128 : (sb + 2) * 128],
                            start=True, stop=True,
                        )
                    PT = ptp.tile([128, 4, 128], BF16, tag="PT")
                    nc.scalar.activation(PT[:], psw[:], mybir.ActivationFunctionType.Exp)
                    # keep 2sp+js <= 2tc+jt: blocks (0,0),(0,1),(1,1) keep
                    # tc >= sp; block (1,0) keeps tc >= sp+1. The two js=0
                    # blocks share the relation -> one 2D-pattern select.
                    nc.gpsimd.affine_select(
                        out=PT[:, 0:2, :], in_=PT[:, 0:2, :],
                        compare_op=mybir.AluOpType.is_ge, fill=0.0,
                        base=0, channel_multiplier=-1,
                        pattern=[[0, 2], [1, 128]],
                    )
                    nc.gpsimd.affine_select(
                        out=PT[:, 2:4, :], in_=PT[:, 2:4, :],
                        compare_op=mybir.AluOpType.is_ge, fill=0.0,
                        base=-1, channel_multiplier=-1,
                        pattern=[[1, 2], [1, 128]],
                    )
                    pts[g].append(PT)

            def attn_out(g):
                b0 = g * GRP
                _, v1 = state[g]
                for b2 in range(GRP):
                    PT = pts[g][b2]
                    sb = b2 * 2
                    # out[t, 0:H] = P @ v ; out[t, H] = rowsum (ones column)
                    pso = psop.tile([128, 2, H + 1], F32, tag="pso")
                    for jt in range(2):
                        for js in range(2):
                            nc.tensor.matmul(
                                pso[:, jt, :],
                                PT[:, 2 * js + jt, :],
                                v1[:, sb + js, :],
                                start=(js == 0), stop=(js == 1),
                            )
                    rec = outp.tile([128, 2, 1], F32, tag="rec")
                    nc.vector.reciprocal(rec[:], pso[:, :, H : H + 1])
                    ob = outp.tile([128, 2, H], F32, tag="ob")
                    for jt in range(2):
                        nc.vector.tensor_scalar_mul(
                            ob[:, jt, :], pso[:, jt, 0:H], rec[:, jt, :]
                        )
                    nc.sync.dma_start(
                        y[b0 + b2].rearrange("(p j) h -> p j h", j=2),
                        ob[:],
                    )
                del state[g], pts[g]

            # transposes run one group ahead of projections so the scalar
            # PSUM->SBUF copies have a full iteration of slack before
            # psqk(g) consumes xT(g).
            LOOKAHEAD = 4
            xbs = {g: xb for g, xb in prologue_loads}
            xTs = {0: pe_transpose(0, xbs.pop(0))}
            for g in range(ngrp):
                if g + LOOKAHEAD < ngrp:
                    xbs[g + LOOKAHEAD] = load_x(g + LOOKAHEAD)
                if g >= 1:
                    attn_logits(g - 1)
                if g + 1 < ngrp:
                    xTs[g + 1] = pe_transpose(g + 1, xbs.pop(g + 1))
                project(g, xTs.pop(g))
                if g >= 1:
                    attn_out(g - 1)
            attn_logits(ngrp - 1)
            attn_out(ngrp - 1)

    nc.finalize()
    return nc


_NC_CACHE = {}


def _get_nc(bpc: int = BPC):
    if bpc not in _NC_CACHE:
        _NC_CACHE[bpc] = build_kernel(bpc)
    return _NC_CACHE[bpc]


def kernel(x, Wk, Wq, Wv, _trace: bool = False, _bpc: int = BPC):
    """Full inputs in, full output out. Shards batch dim over 8 cores."""
    x = np.ascontiguousarray(x, dtype=np.float32)
    Wk = np.ascontiguousarray(Wk, dtype=np.float32)
    Wq = np.ascontiguousarray(Wq, dtype=np.float32)
    Wv = np.ascontiguousarray(Wv, dtype=np.float32)
    nb = x.shape[0]
    bpc = nb // N_CORES
    nc = _get_nc(bpc)
    in_maps = [
        {"x": x[i * bpc : (i + 1) * bpc], "wq": Wq, "wk": Wk, "wv": Wv}
        for i in range(N_CORES)
    ]
    res = run_bass_kernel_spmd(
        nc, in_maps, core_ids=list(range(N_CORES)), trace=_trace
    )
    out = np.concatenate([res.results[i]["y"] for i in range(N_CORES)], axis=0)
    if _trace:
        kernel.last_results = res
    return out
